# revision 4
# baseline (speedup 1.0000x reference)
"""DeeperGCN forward on 8 TRN2 NeuronCores — v2 (dma_gather batched).

Math (exact algebraic collapse of the reference):
  m_e = relu(feat[src]);  per (dst node n, dim d):
    den[n,d] = sum_e exp(m_e),  num[n,d] = sum_e m_e*exp(m_e)
    agg[n,d] = num/den  (0 for isolated nodes)
  out = f(mean(feat,0), S) with S[d] = sum_n agg[n,d]  (host epilogue).

Device: per core 50 dst-node windows of 128 nodes (rank-matched so the SPMD
schedule is core-invariant). Edge messages m are host-relu'd bf16 rows of
featR, fetched with dma_gather (SWDGE, <=1024 rows/op, int16 idx => lo/hi
src split at 32768). Per 128-edge tile: one-hot(iota==dstl) matmul
accumulates [den|num] into the window's PSUM slot; per group of GW windows a
strided epilogue computes ratio = num * exp(-ln(den+eps)) (Ln+Exp share one
ACT table set) and an identity matmul accumulates ratios into a PSUM acc.
Final ones-matmul reduces partitions -> S partial [128] per core.
"""
import math

import numpy as np

P = 128
N_NODES = 50000
N_EDGES = 800000
D = 128
N_CORES = 8
NWIN = 400
NSLOT = NWIN // N_CORES          # 50
GW = 5                           # window slots per group (5 psum banks, bufs=1)
PSL = 512                        # psum cols per slot (full 2KB bank: no sharing)
KCYC = 2                         # groups per deferred-epilogue cycle
SPLIT = 32768                    # lo: src < SPLIT (int16 idx limit)
GOP = 8                          # max tiles (1024 idxs) per dma_gather op
EPS = 1e-6

_CACHE = {}


def _schedule(TL, TH):
    """Static per-core schedule shared by all cores (derived from TL/TH)."""
    NG = (NSLOT + GW - 1) // GW
    groups = []
    t0 = 0
    for g in range(NG):
        slots = list(range(g * GW, min((g + 1) * GW, NSLOT)))
        segs = []                      # (cls, slot, t0, ntiles)
        for cl, TX in ((0, TL), (1, TH)):
            for j in slots:
                segs.append((cl, j, t0, TX[j]))
                t0 += TX[j]
        groups.append((slots, segs))
    return NG, groups, t0


def _build(TL, TH, reps=1):
    import concourse.bacc as bacc
    import concourse.tile as tile
    from concourse import bass, mybir

    f32 = mybir.dt.float32
    bf16 = mybir.dt.bfloat16
    i16 = mybir.dt.int16
    AF = mybir.ActivationFunctionType

    NG, groups, TT = _schedule(TL, TH)
    live = [j for j in range(NSLOT) if TL[j] + TH[j] > 0]
    first_live, last_live = live[0], live[-1]

    nc = bacc.Bacc("TRN2", target_bir_lowering=False, debug=False,
                   num_devices=N_CORES, dynamic_dma_scratch_size=32768)
    featR_d = nc.dram_tensor("featR", [N_NODES, D], bf16, kind="ExternalInput")
    idx_d = nc.dram_tensor("idx16", [P, TT * 8], i16, kind="ExternalInput")
    dstl_d = nc.dram_tensor("dstl", [P, TT], f32, kind="ExternalInput")
    outp_d = nc.dram_tensor("outp", [P, 1], f32, kind="ExternalOutput")

    with tile.TileContext(nc) as tc:
        with tc.tile_pool(name="cst", bufs=1) as cst, \
             tc.tile_pool(name="g", bufs=2) as gp, \
             tc.tile_pool(name="pq", bufs=2) as rp, \
             tc.tile_pool(name="oh", bufs=12) as ohp, \
             tc.tile_pool(name="ep", bufs=1) as epp, \
             tc.tile_pool(name="st", bufs=2) as stp, \
             tc.tile_pool(name="ps", bufs=1, space="PSUM") as psp, \
             tc.tile_pool(name="acc", bufs=1, space="PSUM") as accp, \
             tc.tile_pool(name="fr", bufs=1, space="PSUM") as frp:

            idx16 = cst.tile([P, TT * 8], i16)
            nc.gpsimd.dma_start(idx16[:, 0:64], idx_d.ap()[:, 0:64])
            nc.gpsimd.dma_start(idx16[:, 64:], idx_d.ap()[:, 64:])
            dstl = cst.tile([P, TT], f32)
            nc.sync.dma_start(dstl[:], dstl_d.ap())
            # warm the Q7 gather path during the idx16 load: the first
            # dma_gather after an idle period pays a ~25us wakeup.
            widx = cst.tile([P, 1], i16)
            nc.gpsimd.iota(widx[:], pattern=[[1, 1]], base=0,
                           channel_multiplier=0,
                           allow_small_or_imprecise_dtypes=True)
            wout = cst.tile([P, D], bf16)
            nc.gpsimd.dma_gather(
                wout[:].rearrange("p (t d) -> p t d", d=D),
                featR_d.ap(), widx[:, 0:1], 16, 16, D)
            iota = cst.tile([P, P], bf16)
            nc.gpsimd.iota(iota[:], pattern=[[1, P]], base=0,
                           channel_multiplier=0,
                           allow_small_or_imprecise_dtypes=True)
            pidx = cst.tile([P, 1], f32)
            nc.gpsimd.iota(pidx[:], pattern=[[1, 1]], base=0,
                           channel_multiplier=1,
                           allow_small_or_imprecise_dtypes=True)
            ident = cst.tile([P, P], bf16)
            nc.vector.tensor_scalar(out=ident[:], in0=iota[:],
                                    scalar1=pidx[:, 0:1], scalar2=None,
                                    op0=mybir.AluOpType.is_equal)
            ones = cst.tile([P, 1], f32)
            nc.vector.memset(ones[:], 1.0)
            epsb = cst.tile([P, 1], f32)
            nc.vector.memset(epsb[:], EPS)
            negone = cst.tile([P, 1], f32)
            nc.vector.memset(negone[:], -1.0)

            acc = accp.tile([P, 512], f32)

            for rep in range(reps):
                stage = None
                nstaged = 0
                stage_js = []
                for gi, (slots, segs) in enumerate(groups):
                    Wg = sum(s[3] for s in segs)
                    if Wg == 0:
                        continue
                    gt = gp.tile([P, Wg * D], bf16, tag="gt")
                    # gather: chunks of <=GOP tiles per (class-contiguous) run
                    off = 0
                    for cl in (0, 1):
                        csegs = [s for s in segs if s[0] == cl]
                        seg_tiles = sum(s[3] for s in csegs)
                        if seg_tiles == 0:
                            continue
                        t0g = csegs[0][2]
                        in_ap = (featR_d.ap() if cl == 0
                                 else featR_d.ap()[SPLIT:N_NODES, :])
                        for a in range(0, seg_tiles, GOP):
                            k = min(GOP, seg_tiles - a)
                            out_ap = gt[:, (off + a) * D:(off + a + k) * D] \
                                .rearrange("p (t d) -> p t d", d=D)
                            nc.gpsimd.dma_gather(
                                out_ap, in_ap,
                                idx16[:, (t0g + a) * 8:(t0g + a + k) * 8],
                                k * P, k * P, D)
                        off += seg_tiles

                    pq = rp.tile([P, 2 * Wg * D], bf16, tag="pq")
                    TLg = sum(s_[3] for s_ in segs if s_[0] == 0)
                    CH = 2 * GOP
                    for c0, c1 in ((0, TLg), (TLg, Wg)):
                        for a in range(c0, c1, CH):
                            b = min(a + CH, c1)
                            nc.scalar.activation(pq[:, a * D:b * D],
                                                 gt[:, a * D:b * D], AF.Exp)
                            nc.vector.tensor_tensor(
                                out=pq[:, (Wg + a) * D:(Wg + b) * D],
                                in0=gt[:, a * D:b * D],
                                in1=pq[:, a * D:b * D],
                                op=mybir.AluOpType.mult)
                    pqv = pq[:, :].rearrange("p (h c) -> p h c", h=2)

                    ps = psp.tile([P, GW * PSL], f32, tag="ps")
                    tiles = []
                    for cl, j, t0g, Tn in segs:
                        for t in range(Tn):
                            tiles.append((
                                j - slots[0], t0g + t,
                                (t == 0) and (cl == 0 or TL[j] == 0),
                                (t == Tn - 1) and (cl == 1 or TH[j] == 0)))
                    OHLA = 8
                    ohq = []
                    for lt in range(len(tiles)):
                        while len(ohq) <= OHLA and len(ohq) + lt < len(tiles):
                            gtile = tiles[lt + len(ohq)][1]
                            ohw = ohp.tile([P, P], bf16, tag="ohw")
                            nc.vector.tensor_scalar(
                                out=ohw[:], in0=iota[:],
                                scalar1=dstl[:, gtile:gtile + 1], scalar2=None,
                                op0=mybir.AluOpType.is_equal)
                            ohq.append(ohw)
                        jj, gtile, start, stop = tiles[lt]
                        ohw = ohq.pop(0)
                        nc.tensor.matmul(
                            ps[:, jj * PSL:jj * PSL + 256], lhsT=ohw[:],
                            rhs=pqv[:, :, lt * P:(lt + 1) * P],
                            start=start, stop=stop)

                    # stage den|num (PSUM -> SBUF) with Copy (same ACT
                    # table set as Exp); defer recip epilogue KCYC groups
                    nlive = sum(1 for j in slots if TL[j] + TH[j] > 0)
                    if nlive == 0:
                        continue
                    if stage is None:
                        stage = stp.tile([P, KCYC * GW * 256], bf16, tag="stage")
                        nstaged = 0
                        stage_js = []
                    ps_r = ps[:, 0:nlive * PSL].rearrange(
                        "p (j x) -> p j x", x=PSL)
                    st_r = stage[:, nstaged * 256:(nstaged + nlive) * 256] \
                        .rearrange("p (j x) -> p j x", x=256)
                    nc.scalar.activation(st_r[:, :, 0:P], ps_r[:, :, 0:P],
                                         AF.Copy)
                    nc.scalar.activation(st_r[:, :, P:256], ps_r[:, :, P:256],
                                         AF.Copy)
                    nstaged += nlive
                    stage_js.extend(slots[0] + jj for jj in range(nlive))
                    if gi % KCYC == KCYC - 1 or gi == len(groups) - 1:
                        sf = stage[:, 0:nstaged * 256].rearrange(
                            "p (j x) -> p j x", x=256)
                        lnt = epp.tile([P, nstaged * P], f32, tag="lnt")
                        nc.scalar.activation(lnt[:], sf[:, :, 0:P], AF.Ln,
                                             bias=epsb[:, 0:1])
                        nc.scalar.activation(lnt[:], lnt[:], AF.Exp,
                                             scale=negone[:, 0:1])
                        ratio = epp.tile([P, nstaged * P], bf16, tag="ratio")
                        nc.vector.tensor_tensor(out=ratio[:],
                                                in0=sf[:, :, P:256],
                                                in1=lnt[:],
                                                op=mybir.AluOpType.mult)
                        for si, j in enumerate(stage_js):
                            nc.tensor.matmul(
                                acc[:, 0:P], lhsT=ident[:],
                                rhs=ratio[:, si * P:(si + 1) * P],
                                start=(j == first_live), stop=(j == last_live))
                        stage = None

                acc_sb = epp.tile([P, P], f32, tag="accsb")
                nc.scalar.copy(acc_sb[:], acc[:, 0:P])
                fred = frp.tile([P, 512], f32)
                nc.tensor.matmul(fred[:, 0:1], lhsT=acc_sb[:], rhs=ones[:],
                                 start=True, stop=True)
                outsb = epp.tile([P, 1], f32, tag="outsb")
                nc.scalar.copy(outsb[:], fred[:, 0:1])
                nc.sync.dma_start(outp_d.ap(), outsb[:])

    nc.compile()
    return nc


def _preprocess(feat, src, dst):
    import ml_dtypes

    src = np.ascontiguousarray(src, dtype=np.int64)
    dst = np.ascontiguousarray(dst, dtype=np.int64)
    feat32 = np.ascontiguousarray(feat, dtype=np.float32)

    featR = np.maximum(feat32, 0.0).astype(ml_dtypes.bfloat16)
    feat_sum = feat32.sum(axis=0, dtype=np.float64)

    win = dst >> 7
    loc = (dst & 127).astype(np.float32)
    cls = (src >= SPLIT).astype(np.int64)
    key = win * 2 + cls
    order = np.argsort(key, kind="stable")
    src_s = src[order]
    loc_s = loc[order]
    cnt2 = np.bincount(key, minlength=NWIN * 2).reshape(NWIN, 2)
    starts = np.zeros(NWIN * 2 + 1, np.int64)
    np.cumsum(cnt2.ravel(), out=starts[1:])

    tot = cnt2.sum(1)
    worder = np.argsort(-tot, kind="stable")
    bags = worder.copy().reshape(NSLOT, N_CORES)
    lo_c = cnt2[:, 0]
    hi_c = cnt2[:, 1]

    def _slot_cost(b):
        return (-(-int(lo_c[b].max()) // P)) + (-(-int(hi_c[b].max()) // P))

    cost = np.array([_slot_cost(bags[j]) for j in range(NSLOT)])
    # local search: swap windows between bags to cut ceil-padding
    rs = np.random.RandomState(0)
    for _ in range(120000):
        j1, j2 = rs.randint(0, NSLOT), rs.randint(0, NSLOT)
        if j1 == j2:
            continue
        i1, i2 = rs.randint(0, N_CORES), rs.randint(0, N_CORES)
        bags[j1, i1], bags[j2, i2] = bags[j2, i2], bags[j1, i1]
        c1, c2 = _slot_cost(bags[j1]), _slot_cost(bags[j2])
        if c1 + c2 < cost[j1] + cost[j2]:
            cost[j1], cost[j2] = c1, c2
        else:
            bags[j1, i1], bags[j2, i2] = bags[j2, i2], bags[j1, i1]

    assign = np.zeros((N_CORES, NSLOT), np.int64)
    TL = np.zeros(NSLOT, np.int64)
    TH = np.zeros(NSLOT, np.int64)
    for j in range(NSLOT):
        grp = bags[j]
        assign[:, j] = grp
        TL[j] = int(math.ceil(cnt2[grp, 0].max() / P))
        TH[j] = int(math.ceil(cnt2[grp, 1].max() / P))
    # serpentine-balance slots across groups so every group has a similar
    # tile count (bounds gt/pq SBUF tiles and evens pipeline periods)
    NGx = (NSLOT + GW - 1) // GW
    w = TL + TH
    order2 = np.argsort(-w, kind="stable")
    buckets = [[] for _ in range(NGx)]
    for i, sj in enumerate(order2):
        r = i // NGx
        k = i % NGx
        g = k if r % 2 == 0 else NGx - 1 - k
        buckets[g].append(int(sj))
    perm = [sj for b in buckets for sj in b]
    TL = TL[perm]
    TH = TH[perm]
    assign = assign[:, perm]
    TL = tuple(int(x) for x in TL)
    TH = tuple(int(x) for x in TH)

    NG, groups, TT = _schedule(TL, TH)

    idx_flat = np.zeros((N_CORES, TT * P), np.int64)
    dstl = np.full((N_CORES, P, TT), -1.0, np.float32)
    for c in range(N_CORES):
        for slots, segs in groups:
            for cl, j, t0s, Tn in segs:
                if Tn == 0:
                    continue
                w = assign[c, j]
                s0 = starts[w * 2 + cl]
                n = int(cnt2[w, cl])
                sl = src_s[s0:s0 + n] - (SPLIT if cl else 0)
                ll = loc_s[s0:s0 + n]
                idx_flat[c, t0s * P:t0s * P + n] = sl
                lb = np.full(Tn * P, -1.0, np.float32)
                lb[:n] = ll
                dstl[c, :, t0s:t0s + Tn] = lb.reshape(Tn, P).T

    # wrap to [16, TT*8] then replicate x8 down partitions (per Q7 subcore)
    idx16 = np.zeros((N_CORES, P, TT * 8), np.int16)
    for c in range(N_CORES):
        blk = idx_flat[c].reshape(-1, 16).T.astype(np.int16)
        idx16[c] = np.tile(blk, (8, 1))
    dstl16 = dstl

    return (TL, TH), featR, idx16, dstl16, feat_sum


def kernel(feat, src, dst, Wl, bl, Wout, bout):
    from concourse.bass_utils import run_bass_kernel_spmd

    keyTLTH, featR, idx16, dstl16, feat_sum = _preprocess(feat, src, dst)
    if keyTLTH not in _CACHE:
        _CACHE[keyTLTH] = _build(*keyTLTH)
    nc = _CACHE[keyTLTH]

    in_maps = [
        {"featR": featR, "idx16": idx16[c], "dstl": dstl16[c]}
        for c in range(N_CORES)
    ]
    res = run_bass_kernel_spmd(nc, in_maps, core_ids=list(range(N_CORES)))

    S = np.zeros(D, np.float64)
    for c in range(N_CORES):
        S += res.results[c]["outp"][:, 0].astype(np.float64)

    mean_feat = (feat_sum / N_NODES).astype(np.float32)
    mean_agg = (S / N_NODES).astype(np.float32)
    mean_base = mean_feat + mean_agg
    Wsum = np.asarray(Wl, np.float32).sum(axis=0)
    bsum = np.asarray(bl, np.float32).sum(axis=0)
    h = mean_feat + mean_base @ Wsum + bsum
    out = h @ np.asarray(Wout, np.float32) + np.asarray(bout, np.float32)
    return out[None, :].astype(np.float32)

